# revision 9
# baseline (speedup 1.0000x reference)
"""Trainium2 Bass kernel for nn_EternalNeuralLayer.

Math: out = tanh(x @ W_c + b_c + probs[None, :]) where
probs[j] = |state[j, 0]|^2 after 27 nearest-neighbour circulant "gates"
applied to the uniform state 1/sqrt(n). Each gate matrix
G = cos*I - sin*P + sin*P^T is circulant, and the uniform vector is its
eigenvector with eigenvalue cos(theta), so the state stays uniform:
probs[j] = (prod_{d,g} cos(ew[d, j, g]))^2 / n   (g in 0..2, d in 0..8).

Sharding: data-parallel over the batch (8 cores x 512 rows). Every core
streams the full classical_weights [2048, 2048] and computes its
x-shard's GEMM as outT[m, b] = sum_k W[k, m] * xT[k, b] (output m on
partitions so the per-output bias (b_c + probs) is a per-partition ACT
bias), applies tanh on the Scalar engine directly out of PSUM, and
writes its outT shard. The eternal-probs product is computed on-device
per core from the [27, 2048] angle slice (tiny). No collectives.

GEMM precision: main pass xh @ Wh in float32r (fp32 with 11 explicit
mantissa bits, full PE rate, operands pre-rounded host-side). The two
rounding-residual corrections run as ONE fp8e5 (e5m2) DoubleRow stream:
a DoubleRow matmul computes Wh8.T @ xl8 + Wl8.T @ xh8 (two independent
K=128 plane products) at 0.5 cycles/output-column -- 4x the fp32r MAC
rate. e5m2's 2^-15 dynamic range holds the ~2^-12-scale residuals
unscaled, so the corrections accumulate into the SAME PSUM bank as the
main pass and the epilogue stays a single fused tanh. End-to-end absmax
vs the fp32 reference ~3e-3 (vs 2.7e-2 for the uncorrected fp32r pass).

PE work per core: 16 m-tiles x (16 fp32r matmuls @512 cols + 32
DoubleRow matmuls @256 cols) = 196.6k cycles = 82 us at 2.4 GHz, vs
164 us for the previous 3-pass fp32r hi/lo scheme.

Per-tile instruction order is a kb-wave: [DR(c0), DR(c1), main] per
k-tile; the first DR carries start=True (PSUM pending-zero covers the
whole bank) and the last main closes the group. The first GR0 m-tiles'
waves are interleaved so the DMA-paced ramp keeps the PE dense.
DMA rings: fp32r xt/w + ang on the sync ring, fp8 x8/w8 on the vector
ring (concurrent head streams), output stores on the scalar ring.
"""

import math
import os
import sys

import numpy as np
import ml_dtypes

for _p in ("/opt/trn_rl_repo", "/root/.axon_site/_ro/trn_rl_repo"):
    if _p not in sys.path and os.path.isdir(_p):
        sys.path.append(_p)

import concourse.bass as bass  # noqa: E402
import concourse.tile as tile  # noqa: E402
from concourse import bacc, mybir  # noqa: E402
from concourse.bass_utils import run_bass_kernel_spmd  # noqa: E402

N_CORES = 8
B, N, M, D = 4096, 2048, 2048, 9
BS = B // N_CORES  # 512 batch rows per core
KT = N // 128  # 16 contraction tiles
MT = M // 128  # 16 output m-tiles
MG = 2  # m-tiles per output DMA group
GR0 = 6  # ramp-interleaved head m-tiles (each pinned to its own PSUM bank)
WPRE = 6  # W-tile prefetch depth
NGATE = D * 3  # 27 rotation gates
GPAD = 32  # padded gate slots (pad with 0.0 -> cos = 1)

F32 = mybir.dt.float32
F32R = mybir.dt.float32r
F8 = mybir.dt.float8e5
BF16 = mybir.dt.bfloat16
DR = mybir.MatmulPerfMode.DoubleRow


def build_program():
    nc = bacc.Bacc(
        "TRN2", target_bir_lowering=False, debug=False, num_devices=N_CORES
    )
    # xt[p, kb*BS + b] = xh[b, kb*128 + p]  (fp32r high part of x)
    xt_d = nc.dram_tensor("xt", [128, KT * BS], F32R, kind="ExternalInput").ap()
    # x8[p, kb, 0, b] = e5m2(xl)[b, kb*128+p]; [.., 1, b] = e5m2(xh)[b, ..]
    x8_d = nc.dram_tensor("x8", [128, KT, 2, BS], F8, kind="ExternalInput").ap()
    # w[t*128 + p, kb*128 + m] = Wh[kb*128 + p, t*128 + m]  (fp32r)
    w_d = nc.dram_tensor("w", [M, N], F32R, kind="ExternalInput").ap()
    # w8[t*128+p, kb, 0, m] = e5m2(Wh)[kb*128+p, t*128+m]; [..,1,m] = e5m2(Wl)
    w8_d = nc.dram_tensor("w8", [M, KT, 2, 128], F8, kind="ExternalInput").ap()
    ang_d = nc.dram_tensor("ang", [128, GPAD * MT], F32, kind="ExternalInput").ap()
    cbt_d = nc.dram_tensor("cbt", [128, MT], F32, kind="ExternalInput").ap()
    # out_dev[g, ml, j*BS + b] = tanh(...)[m = (g*MG+j)*128 + ml, b]
    # bf16: tanh output is in [-1, 1], so bf16 adds <= 2^-9 abs error and
    # halves the store traffic; host_post upconverts to fp32.
    out_d = nc.dram_tensor(
        "out_dev", [MT // MG, 128, MG * BS], BF16, kind="ExternalOutput"
    ).ap()

    with tile.TileContext(nc) as tc:
        with (
            tc.tile_pool(name="xt", bufs=1) as xt_pool,
            tc.tile_pool(name="x8", bufs=1) as x8_pool,
            tc.tile_pool(name="w", bufs=WPRE) as w_pool,
            tc.tile_pool(name="w8", bufs=WPRE) as w8_pool,
            tc.tile_pool(name="ps", bufs=3, space="PSUM") as ps_pool,
            tc.tile_pool(name="out", bufs=3) as out_pool,
            tc.tile_pool(name="small", bufs=1) as small_pool,
        ):
            # --- GEMM input DMAs, all on the sync HWDGE ring so arrival
            # ORDER is exactly the issue order (one ring saturates the
            # ~343 GB/s core bandwidth by itself). Head order feeds the
            # stream-split ramp: fp8 w8/x8 first (corrections run first),
            # then fp32r w/xt. Output stores use the scalar ring so a
            # store waiting on ACT never head-of-line-blocks loads. ---
            wts = {}
            w8ts = {}

            def fetch_w(t):
                wt = w_pool.tile([128, KT * 128], F32R, tag="w")
                nc.sync.dma_start(wt[:], w_d[t * 128 : (t + 1) * 128, :])
                wts[t] = wt

            def fetch_w8(t):
                wt8 = w8_pool.tile([128, KT, 2, 128], F8, tag="w8")
                nc.sync.dma_start(wt8[:], w8_d[t * 128 : (t + 1) * 128])
                w8ts[t] = wt8

            xts = []

            def fetch_xt(s):
                xtk = xt_pool.tile([128, BS], F32R, tag=f"xt{s}")
                nc.sync.dma_start(xtk[:], xt_d[:, s * BS : (s + 1) * BS])
                xts.append(xtk)

            # one fp8 x tile; fetched in 4-ktile chunks
            x8t = x8_pool.tile([128, KT, 2, BS], F8, name="x8t")

            def fetch_x8(c):  # c in 0..3
                nc.sync.dma_start(
                    x8t[:, 4 * c : 4 * (c + 1)], x8_d[:, 4 * c : 4 * (c + 1)]
                )

            # fp8 head: w8 tiles for the ramp group + x8 chunks
            fetch_w8(0)
            fetch_x8(0)
            for g in range(1, GR0):
                fetch_w8(g)
                if g <= 3:
                    fetch_x8(g)
            # fp32r stream: w0..5 interleaved with the xt slabs
            fetch_w(0)
            for s in range(KT):
                fetch_xt(s)
                if s in (1, 4, 7, 10):
                    fetch_w(1 + (s - 1) // 3)
            fetch_w(5)

            # --- eternal probs -> per-output bias [128, MT] (gates only
            # the first epilogue; issued after the GEMM-critical DMAs) ---
            ang = small_pool.tile([128, GPAD * MT], F32)
            nc.sync.dma_start(ang[:], ang_d[:])
            cbt = small_pool.tile([128, MT], F32)
            nc.sync.dma_start(cbt[:], cbt_d[:])

            cosa = small_pool.tile([128, GPAD * MT], F32)
            # cos(a) = sin(a + pi/2); wrap into ACT Sin's [-pi, pi] domain
            # (|a| < 3pi/2 + pi holds for randn angles).
            nc.vector.add_range_wrap(
                cosa[:], ang[:], shift=math.pi / 2, bound=math.pi,
                period=2 * math.pi,
            )
            nc.scalar.activation(
                cosa[:], cosa[:], mybir.ActivationFunctionType.Sin
            )
            # tree-product over the 32 gate slots -> [128, MT]
            half = GPAD * MT // 2
            while half >= MT:
                nc.vector.tensor_mul(
                    cosa[:, 0:half], cosa[:, 0:half], cosa[:, half : 2 * half]
                )
                half //= 2
            bias_t = small_pool.tile([128, MT], F32)
            # probs = (prod cos)^2 / n
            nc.scalar.activation(
                bias_t[:],
                cosa[:, 0:MT],
                mybir.ActivationFunctionType.Square,
                scale=1.0 / math.sqrt(N),
            )
            nc.vector.tensor_add(bias_t[:], bias_t[:], cbt[:])

            # --- column-parallel GEMM over 16 m-tiles ---
            ot_box = [None]

            def epilogue(t, ps):
                j = t % MG
                if j == 0:
                    ot_box[0] = out_pool.tile([128, MG * BS], BF16, name="ot", tag="ot")
                ot = ot_box[0]
                nc.scalar.activation(
                    ot[:, j * BS : (j + 1) * BS],
                    ps[:],
                    mybir.ActivationFunctionType.Tanh,
                    bias=bias_t[:, t : t + 1],
                )
                if j == MG - 1:
                    nc.scalar.dma_start(out_d[t // MG], ot[:])

            def dr_mm(t, ps, kb):
                """Wide fp8e5 DoubleRow correction matmul for one k-tile:
                rhs free 1024 -> all 512 out cols (hw accepts >512 moving
                for fp8). The kb=0 DR opens the PSUM group (pending-zero
                covers the whole bank)."""
                first = kb == 0
                nc.tensor.matmul(
                    ps[:],
                    lhsT=w8ts[t][:, kb, :, :],
                    rhs=x8t[:, kb, :, :],
                    start=first, stop=False,
                    perf_mode=DR,
                    skip_group_check=not first,
                )

            def main_mm(t, ps, kb):
                """fp32r main matmul for one k-tile; kb=KT-1 closes the
                accumulation group."""
                last = kb == KT - 1
                nc.tensor.matmul(
                    ps[:],
                    lhsT=wts[t][:, kb * 128 : (kb + 1) * 128],
                    rhs=xts[kb][:],
                    start=False, stop=last,
                    skip_group_check=not last,
                )

            # ramp, stream-split across the first GR0 m-tiles: all their
            # fp8 DR correction waves first (gated only on the small w8/x8
            # head), then all fp32r main waves (gated on the w/xt stream).
            # kb-major so each arriving slab feeds GR0 tiles of PE work.
            pss = [
                ps_pool.tile([128, BS], F32, name=f"psg{g}", tag=f"psg{g}", bufs=1)
                for g in range(GR0)
            ]
            for kb in range(KT):
                for g in range(GR0):
                    dr_mm(g, pss[g], kb)
            for kb in range(KT):
                for g in range(GR0):
                    main_mm(g, pss[g], kb)
            for g in range(GR0):
                wts.pop(g)
                w8ts.pop(g)
                epilogue(g, pss[g])
                if g + GR0 < MT:
                    fetch_w8(g + GR0)
                    fetch_w(g + GR0)

            for t in range(GR0, MT):
                ps = ps_pool.tile([128, BS], F32, tag="ps", bufs=2)
                for kb in range(KT):
                    dr_mm(t, ps, kb)
                for kb in range(KT):
                    main_mm(t, ps, kb)
                wts.pop(t)
                w8ts.pop(t)
                tn = t + WPRE
                if tn < MT:
                    fetch_w8(tn)
                    fetch_w(tn)
                epilogue(t, ps)

    nc.compile()
    return nc


def to_fp32r(a):
    """Round fp32 -> fp32r storage (1-8-11 float in the top 20 bits, i.e.
    fp32 with the low 12 mantissa bits zeroed, round-to-nearest-even)."""
    u = np.ascontiguousarray(a, dtype=np.float32).view(np.uint32).astype(np.uint64)
    lsb = (u >> 12) & 1
    u = (u + 0x7FF + lsb) & 0xFFFFF000
    return u.astype(np.uint32).view(np.float32)


def _e5(a):
    return np.asarray(a, dtype=np.float32).astype(ml_dtypes.float8_e5m2)


def _relayout_w(w):
    """[N, M] -> w_dev[t*128 + p, kb*128 + m] = w[kb*128 + p, t*128 + m]
    so each m-tile's [128, N] slab is row-contiguous."""
    return w.reshape(KT, 128, MT, 128).transpose(2, 1, 0, 3).reshape(M, N)


def host_prep(x, eternal_weights, classical_weights, classical_biases):
    """Shard + lay out the inputs for the 8 cores (DMA-friendly layouts)."""
    x = np.ascontiguousarray(x, dtype=np.float32)
    w = np.ascontiguousarray(classical_weights, dtype=np.float32)
    cb = np.asarray(classical_biases, dtype=np.float32)

    xh = to_fp32r(x)
    wh = to_fp32r(w)
    w_dev = np.ascontiguousarray(_relayout_w(wh))

    # fp8 planes: plane0 pairs e5m2(Wh) with e5m2(xl), plane1 pairs
    # e5m2(Wl) with e5m2(xh)
    wh8 = _e5(wh)
    wl8 = _e5((w - wh).astype(np.float32))
    # w8_dev[t*128+p, kb, pl, m] = W8pl[kb*128+p, t*128+m]
    w8 = np.stack([wh8, wl8], axis=0).reshape(2, KT, 128, MT, 128)
    w8_dev = np.ascontiguousarray(
        w8.transpose(3, 2, 1, 0, 4).reshape(M, KT, 2, 128)
    )

    # angles actually used: [D, M, 3] -> [27, M]; device layout
    # ang[p, g*MT + t] = angle_g[t*128 + p], zero-padded to GPAD slots.
    a = np.transpose(np.asarray(eternal_weights[:, :M, :3], dtype=np.float32),
                     (0, 2, 1)).reshape(NGATE, M)
    ar = a.reshape(NGATE, MT, 128)  # [g, t, p]
    ang = np.zeros((128, GPAD, MT), dtype=np.float32)
    ang[:, :NGATE, :] = np.transpose(ar, (2, 0, 1))
    ang = np.ascontiguousarray(ang.reshape(128, GPAD * MT))

    cbt = np.ascontiguousarray(cb.reshape(MT, 128).T)  # [128, MT]

    def shard_xt(xs):
        # [BS, N] -> [128, KT, BS]: xt[p, kb, b] = xs[b, kb*128 + p]
        return xs.reshape(BS, KT, 128).transpose(2, 1, 0)

    in_maps = []
    for c in range(N_CORES):
        sl = slice(c * BS, (c + 1) * BS)
        xt = np.ascontiguousarray(shard_xt(xh[sl]).reshape(128, KT * BS))
        xl8 = _e5((x[sl] - xh[sl]).astype(np.float32))
        xh8 = _e5(xh[sl])
        # x8[p, kb, pl, b]
        x8 = np.stack([xl8, xh8], axis=0).reshape(2, BS, KT, 128)
        x8 = np.ascontiguousarray(x8.transpose(3, 2, 0, 1))
        in_maps.append({
            "xt": xt, "x8": x8, "w": w_dev, "w8": w8_dev,
            "ang": ang, "cbt": cbt,
        })
    return in_maps


def host_post(results):
    """Reassemble [4096, 2048] from the 8 cores' out_dev blocks."""
    parts = []
    for c in range(N_CORES):
        od = np.asarray(results[c]["out_dev"]).astype(np.float32)
        # outT[(g*MG + j)*128 + ml, b] = od[g, ml, j*BS + b]
        outT = (
            od.reshape(MT // MG, 128, MG, BS)
            .transpose(0, 2, 1, 3)
            .reshape(M, BS)
        )
        parts.append(outT.T)  # [BS, M]
    return np.ascontiguousarray(np.concatenate(parts, axis=0), dtype=np.float32)


_NC_CACHE = {}


def _get_program():
    if "nc" not in _NC_CACHE:
        _NC_CACHE["nc"] = build_program()
    return _NC_CACHE["nc"]


def kernel(x, eternal_weights, eternal_biases, classical_weights, classical_biases,
           _trace=False):
    nc = _get_program()
    in_maps = host_prep(x, eternal_weights, classical_weights, classical_biases)
    res = run_bass_kernel_spmd(nc, in_maps, list(range(N_CORES)), trace=_trace)
    out = host_post(res.results)
    if _trace:
        kernel.last_exec_time_ns = res.exec_time_ns
        kernel.last_results = res
    return out


# revision 20
# speedup vs baseline: 1.0504x; 1.0504x over previous
"""Trainium2 Bass kernel for nn_EternalNeuralLayer.

Math: out = tanh(x @ W_c + b_c + probs[None, :]) where
probs[j] = |state[j, 0]|^2 after 27 nearest-neighbour circulant "gates"
applied to the uniform state 1/sqrt(n). Each gate matrix
G = cos*I - sin*P + sin*P^T is circulant, and the uniform vector is its
eigenvector with eigenvalue cos(theta), so the state stays uniform:
probs[j] = (prod_{d,g} cos(ew[d, j, g]))^2 / n   (g in 0..2, d in 0..8).

Sharding: 4x2 grid -- 4-way data parallel over the batch (1024 rows per
core) x 2-way column parallel over the output dim (1024 cols per core).
Core c handles batch shard c%4 and M-half c//4. Each core computes
outT[m, b] = sum_k W[k, m] * xT[k, b] for its [1024, 1024] block
(output m on partitions so the per-output bias (b_c + probs) is a
per-partition ACT bias), applies tanh on the Scalar engine directly out
of PSUM, and stores bf16 (tanh is in [-1, 1]; bf16 adds <= 2^-9).
The eternal-probs product is computed on-device per core from its
[27, 1024] angle slice. No collectives; host stitches 8 blocks.
The 2D grid halves both the W-stream and x-stream per core vs pure
data-parallel, which makes even the DMA-paced ramp nearly PE-bound.

GEMM precision: main pass xh @ Wh in float32r (fp32 with 11 explicit
mantissa bits, full PE rate, operands pre-rounded host-side).
Rounding-residual corrections run as fp8e5 (e5m2) DoubleRow matmuls:
one DoubleRow computes two independent K=128 plane products in 512
cycles -- 2x the fp32r MAC rate (measured; the cost model's 0.5
cyc/row does not materialize on TRN2 silicon in any layout/mode).
e5m2's 2^-15 dynamic range holds the ~2^-12-scale residuals unscaled,
so corrections accumulate into the SAME PSUM bank as the main pass and
the epilogue stays one fused tanh. Correction coverage is partial (see
UNITS): x-residual on all 16 k-tiles, W-residual on 8 -- measured
absmax 1.52e-2 vs the 2e-2 gate (3.1e-3 at full coverage). The error
is fully deterministic: the device result matches the host numpy model
of the exact rounding chain to 5 digits.

Work per core: 8 m-tiles x 2 batch-chunk units of [128m, 512b]; per
unit 12 DoubleRow (512 cyc) + 16 fp32r (512 cyc) matmuls ~= 103 us of
PE at 2.4 GHz. The ramp covers m-tiles 0..2 (6 units): stage A = DR
wavefront + mains wavefront for batch-chunk 0, with batch-chunk-1 DR
matmuls woven into the mains wavefront as DMA-stall filler, then the
chunk-1 mains. All loads ride the sync HWDGE ring in consumption
order; stores use the scalar ring.
"""

import math
import os
import sys

import numpy as np
import ml_dtypes

for _p in ("/opt/trn_rl_repo", "/root/.axon_site/_ro/trn_rl_repo"):
    if _p not in sys.path and os.path.isdir(_p):
        sys.path.append(_p)

import concourse.bass as bass  # noqa: E402
import concourse.tile as tile  # noqa: E402
from concourse import bacc, mybir  # noqa: E402
from concourse.bass_utils import run_bass_kernel_spmd  # noqa: E402

N_CORES = 8
B, N, M, D = 4096, 2048, 2048, 9
GR = 4  # batch shards
GC = 2  # M-half shards
BS = B // GR  # 1024 batch rows per core
MC = M // GC  # 1024 output cols per core
CH = 2  # batch chunks (PSUM units) per m-tile
CW = BS // CH  # 512 batch cols per PSUM unit
KT = N // 128  # 16 contraction tiles
MTC = MC // 128  # 8 m-tiles per core
MTF = M // 128  # 16 m-tiles in the full output
MG = 2  # m-tiles per output DMA group
RT = 3  # ramp m-tiles (RT*CH units pinned to PSUM banks)
NGATE = D * 3  # 27 rotation gates
GPAD = 32  # padded gate slots (pad with 0.0 -> cos = 1)

# Correction coverage. Each DoubleRow "unit" holds two K=128 plane
# products. ("hl", kb) pairs the x-residual with the W-residual for
# k-tile kb: Wh8[kb]*xl8[kb] + Wl8[kb]*xh8[kb]. ("xx", kb) packs two
# k-tiles of the x-residual correction only: Wh8[kb]*xl8[kb] +
# Wh8[kb+1]*xl8[kb+1].
WCOV = 8
UNITS = [("hl", kb) for kb in range(WCOV)] + [
    ("xx", kb) for kb in range(WCOV, KT, 2)
]
NU = len(UNITS)  # 12
NCH = NU // 2  # x8 fetch chunks (2 units each) per batch chunk

F32 = mybir.dt.float32
F32R = mybir.dt.float32r
F8 = mybir.dt.float8e5
BF16 = mybir.dt.bfloat16
DR = mybir.MatmulPerfMode.DoubleRow


def build_program():
    nc = bacc.Bacc(
        "TRN2", target_bir_lowering=False, debug=False, num_devices=N_CORES
    )
    # xt[p, kb, j, b] = xh[j*CW + b, kb*128 + p]  (fp32r high part of x)
    xt_d = nc.dram_tensor("xt", [128, KT, CH, CW], F32R, kind="ExternalInput").ap()
    # x8[p, j, u, pl, b]: correction-unit moving planes per batch chunk
    x8_d = nc.dram_tensor("x8", [128, CH, NU, 2, CW], F8, kind="ExternalInput").ap()
    # w[t*128 + p, kb*128 + m] = Wh[kb*128 + p, t*128 + m]  (fp32r)
    w_d = nc.dram_tensor("w", [MC, N], F32R, kind="ExternalInput").ap()
    # w8[t*128+p, u, pl, m]: correction-unit stationary planes
    w8_d = nc.dram_tensor("w8", [MC, NU, 2, 128], F8, kind="ExternalInput").ap()
    ang_d = nc.dram_tensor("ang", [128, GPAD * MTC], F32, kind="ExternalInput").ap()
    cbt_d = nc.dram_tensor("cbt", [128, MTC], F32, kind="ExternalInput").ap()
    # out_dev[g, ml, sl*BS + j*CW + b] = tanh(...)[(g*MG+sl)*128+ml, ..]
    out_d = nc.dram_tensor(
        "out_dev", [MTC // MG, 128, MG * BS], BF16, kind="ExternalOutput"
    ).ap()

    with tile.TileContext(nc) as tc:
        with (
            tc.tile_pool(name="xt", bufs=1) as xt_pool,
            tc.tile_pool(name="x8", bufs=1) as x8_pool,
            tc.tile_pool(name="w", bufs=6) as w_pool,
            tc.tile_pool(name="w8", bufs=8) as w8_pool,
            tc.tile_pool(name="ps", bufs=3, space="PSUM") as ps_pool,
            tc.tile_pool(name="out", bufs=3) as out_pool,
            tc.tile_pool(name="small", bufs=1) as small_pool,
        ):
            wts = {}
            w8ts = {}

            def fetch_w(t):
                wt = w_pool.tile([128, KT * 128], F32R, tag="w")
                nc.sync.dma_start(wt[:], w_d[t * 128 : (t + 1) * 128, :])
                wts[t] = wt

            def fetch_w8(t):
                wt8 = w8_pool.tile([128, NU, 2, 128], F8, tag="w8")
                nc.sync.dma_start(wt8[:], w8_d[t * 128 : (t + 1) * 128])
                w8ts[t] = wt8

            xt_t = xt_pool.tile([128, KT, CH, CW], F32R, name="xt_t")

            def fetch_xt(kb, j):
                nc.sync.dma_start(xt_t[:, kb, j], xt_d[:, kb, j])

            x8t = x8_pool.tile([128, CH, NU, 2, CW], F8, name="x8t")

            def fetch_x8(j, c):  # unit chunk c of batch chunk j
                nc.sync.dma_start(
                    x8t[:, j, 2 * c : 2 * (c + 1)], x8_d[:, j, 2 * c : 2 * (c + 1)]
                )

            # --- head DMA, consumption order ---
            # stage A DRs: w8_0..2 interleaved with the chunk-0 x8 stream
            wt8_0 = w8_pool.tile([128, NU, 2, 128], F8, tag="w8")
            w8ts[0] = wt8_0
            w8cuts = [0, 3, 6, 9, NU]
            for c in range(4):
                nc.sync.dma_start(
                    wt8_0[:, w8cuts[c] : w8cuts[c + 1]],
                    w8_d[0:128, w8cuts[c] : w8cuts[c + 1]],
                )
                fetch_x8(0, c)
            fetch_w8(1)
            fetch_x8(0, 4)
            fetch_w8(2)
            fetch_x8(0, 5)
            # stage-B DR feed (chunk-1 x8) starts early: woven as filler
            fetch_x8(1, 0)
            fetch_x8(1, 1)
            # stage A mains: w0..2 with the chunk-0 xt slabs; rest of x8j1
            fetch_w(0)
            for kb in range(KT):
                fetch_xt(kb, 0)
                if kb == 1:
                    fetch_w(1)
                    fetch_x8(1, 2)
                elif kb == 4:
                    fetch_w(2)
                    fetch_x8(1, 3)
                elif kb == 7:
                    fetch_x8(1, 4)
                    fetch_x8(1, 5)
                elif kb == 10:
                    fetch_w8(3)
                    fetch_w(3)
            # stage B mains: chunk-1 xt slabs
            for kb in range(KT):
                fetch_xt(kb, 1)

            # --- eternal probs -> per-output bias [128, MTC] ---
            ang = small_pool.tile([128, GPAD * MTC], F32)
            nc.sync.dma_start(ang[:], ang_d[:])
            cbt = small_pool.tile([128, MTC], F32)
            nc.sync.dma_start(cbt[:], cbt_d[:])

            cosa = small_pool.tile([128, GPAD * MTC], F32)
            # cos(a) = sin(a + pi/2); wrap into ACT Sin's [-pi, pi] domain
            # (|a| < 3pi/2 + pi holds for randn angles).
            nc.vector.add_range_wrap(
                cosa[:], ang[:], shift=math.pi / 2, bound=math.pi,
                period=2 * math.pi,
            )
            nc.scalar.activation(
                cosa[:], cosa[:], mybir.ActivationFunctionType.Sin
            )
            half = GPAD * MTC // 2
            while half >= MTC:
                nc.vector.tensor_mul(
                    cosa[:, 0:half], cosa[:, 0:half], cosa[:, half : 2 * half]
                )
                half //= 2
            bias_t = small_pool.tile([128, MTC], F32)
            nc.scalar.activation(
                bias_t[:],
                cosa[:, 0:MTC],
                mybir.ActivationFunctionType.Square,
                scale=1.0 / math.sqrt(N),
            )
            nc.vector.tensor_add(bias_t[:], bias_t[:], cbt[:])

            # --- GEMM over 8 m-tiles x 2 batch chunks ---
            ot_box = [None]

            def epilogue(t, j, ps):
                sl = t % MG
                if sl == 0 and j == 0:
                    ot_box[0] = out_pool.tile(
                        [128, MG * BS], BF16, name="ot", tag="ot"
                    )
                ot = ot_box[0]
                c0 = sl * BS + j * CW
                nc.scalar.activation(
                    ot[:, c0 : c0 + CW],
                    ps[:],
                    mybir.ActivationFunctionType.Tanh,
                    bias=bias_t[:, t : t + 1],
                )
                if sl == MG - 1 and j == CH - 1:
                    g = t // MG
                    if g == MTC // MG - 1:
                        # final group: store in halves so only ~256 KB
                        # trails the last matmul
                        nc.scalar.dma_start(out_d[g, :, 0:BS], ot[:, 0:BS])
                        nc.scalar.dma_start(
                            out_d[g, :, BS : 2 * BS], ot[:, BS : 2 * BS]
                        )
                    else:
                        nc.scalar.dma_start(out_d[g], ot[:])

            def dr_mm(t, j, ps, u):
                """Wide fp8e5 DoubleRow correction matmul for one unit
                (rhs free 1024 -> all 512 out cols; hw accepts >512
                moving for fp8). u=0 opens the PSUM group (pending-zero
                covers the whole bank)."""
                first = u == 0
                nc.tensor.matmul(
                    ps[:],
                    lhsT=w8ts[t][:, u, :, :],
                    rhs=x8t[:, j, u, :, :],
                    start=first, stop=False,
                    perf_mode=DR,
                    skip_group_check=not first,
                )

            def main_mm(t, j, ps, kb):
                last = kb == KT - 1
                nc.tensor.matmul(
                    ps[:],
                    lhsT=wts[t][:, kb * 128 : (kb + 1) * 128],
                    rhs=xt_t[:, kb, j],
                    start=False, stop=last,
                    skip_group_check=not last,
                )

            # --- ramp: m-tiles 0..RT-1, both chunks (RT*CH pinned banks)
            pss = {
                (t, j): ps_pool.tile(
                    [128, CW], F32, name=f"psg{t}{j}", tag=f"psg{t}{j}", bufs=1
                )
                for t in range(RT)
                for j in range(CH)
            }
            # stage A DR wavefront (chunk 0): diag over (tile, x8 chunk)
            for s in range(NCH + RT - 1):
                for t in range(RT):
                    c = s - t
                    if 0 <= c < NCH:
                        dr_mm(t, 0, pss[(t, 0)], 2 * c)
                        dr_mm(t, 0, pss[(t, 0)], 2 * c + 1)
            # stage A mains wavefront with stage-B DRs woven as filler
            bq = [(t, u) for t in range(RT) for u in range(NU)]
            bi = 0
            for s in range(KT + RT - 1):
                for t in range(RT):
                    kb = s - t
                    if 0 <= kb < KT:
                        main_mm(t, 0, pss[(t, 0)], kb)
                if s >= 2:
                    for _ in range(3 if s % 2 else 2):
                        if bi < len(bq):
                            t, u = bq[bi]
                            bi += 1
                            dr_mm(t, 1, pss[(t, 1)], u)
            while bi < len(bq):
                t, u = bq[bi]
                bi += 1
                dr_mm(t, 1, pss[(t, 1)], u)
            # stage B mains wavefront (chunk 1)
            for s in range(KT + RT - 1):
                for t in range(RT):
                    kb = s - t
                    if 0 <= kb < KT:
                        main_mm(t, 1, pss[(t, 1)], kb)
            # ramp epilogues, t-major so output groups complete in order
            for t in range(RT):
                for j in range(CH):
                    epilogue(t, j, pss[(t, j)])
                wts.pop(t)
                w8ts.pop(t)
                tn = t + 4
                if tn < MTC:
                    fetch_w8(tn)
                    fetch_w(tn)

            # --- cruise: m-tiles RT..MTC-1 ---
            for t in range(RT, MTC):
                for j in range(CH):
                    ps = ps_pool.tile([128, CW], F32, tag="ps", bufs=2)
                    for u in range(NU):
                        dr_mm(t, j, ps, u)
                    for kb in range(KT):
                        main_mm(t, j, ps, kb)
                    epilogue(t, j, ps)
                wts.pop(t)
                w8ts.pop(t)
                tn = t + 4
                if tn < MTC:
                    fetch_w8(tn)
                    fetch_w(tn)

    nc.compile()
    return nc


def to_fp32r(a):
    """Round fp32 -> fp32r storage (1-8-11 float in the top 20 bits, i.e.
    fp32 with the low 12 mantissa bits zeroed, round-to-nearest-even)."""
    u = np.ascontiguousarray(a, dtype=np.float32).view(np.uint32).astype(np.uint64)
    lsb = (u >> 12) & 1
    u = (u + 0x7FF + lsb) & 0xFFFFF000
    return u.astype(np.uint32).view(np.float32)


def _e5(a):
    return np.asarray(a, dtype=np.float32).astype(ml_dtypes.float8_e5m2)


def host_prep(x, eternal_weights, classical_weights, classical_biases):
    """Shard + lay out the inputs for the 4x2 core grid."""
    x = np.ascontiguousarray(x, dtype=np.float32)
    w = np.ascontiguousarray(classical_weights, dtype=np.float32)
    cb = np.asarray(classical_biases, dtype=np.float32)

    # --- per M-half tensors (shared by 4 cores each) ---
    halves = []
    a = np.transpose(np.asarray(eternal_weights[:, :M, :3], dtype=np.float32),
                     (0, 2, 1)).reshape(NGATE, M)
    ar = a.reshape(NGATE, MTF, 128)  # [g, t, p]
    for h in range(GC):
        wsh = np.ascontiguousarray(w[:, h * MC : (h + 1) * MC])
        wh = to_fp32r(wsh)
        w_dev = np.ascontiguousarray(
            wh.reshape(KT, 128, MTC, 128).transpose(2, 1, 0, 3).reshape(MC, N)
        )
        wh8 = _e5(wh)
        wl8 = _e5((wsh - wh).astype(np.float32))

        def _rk(v):  # [N, MC] -> [MTC, 128p, KT, 128m]
            return v.reshape(KT, 128, MTC, 128).transpose(2, 1, 0, 3)

        rh, rl = _rk(wh8), _rk(wl8)
        w8u = np.empty((MTC, 128, NU, 2, 128), dtype=wh8.dtype)
        for u, (kind, kb) in enumerate(UNITS):
            w8u[:, :, u, 0] = rh[:, :, kb]
            w8u[:, :, u, 1] = rl[:, :, kb] if kind == "hl" else rh[:, :, kb + 1]
        w8_dev = np.ascontiguousarray(w8u.reshape(MC, NU, 2, 128))

        ang = np.zeros((128, GPAD, MTC), dtype=np.float32)
        ang[:, :NGATE, :] = np.transpose(
            ar[:, h * MTC : (h + 1) * MTC], (2, 0, 1)
        )
        ang = np.ascontiguousarray(ang.reshape(128, GPAD * MTC))
        cbt = np.ascontiguousarray(
            cb[h * MC : (h + 1) * MC].reshape(MTC, 128).T
        )
        halves.append({"w": w_dev, "w8": w8_dev, "ang": ang, "cbt": cbt})

    # --- per batch-shard tensors (shared by 2 cores each) ---
    def _sh(v):  # [BS, N] -> [128p, KT, CH, CW]
        return v.reshape(CH, CW, KT, 128).transpose(3, 2, 0, 1)

    shards = []
    for r in range(GR):
        xs = x[r * BS : (r + 1) * BS]
        xh = to_fp32r(xs)
        xt = np.ascontiguousarray(_sh(xh))
        sl8 = _sh(_e5((xs - xh).astype(np.float32)))
        sh8 = _sh(_e5(xh))
        x8 = np.empty((128, CH, NU, 2, CW), dtype=sl8.dtype)
        for u, (kind, kb) in enumerate(UNITS):
            for j in range(CH):
                x8[:, j, u, 0] = sl8[:, kb, j]
                x8[:, j, u, 1] = sh8[:, kb, j] if kind == "hl" else sl8[:, kb + 1, j]
        shards.append({"xt": xt, "x8": np.ascontiguousarray(x8)})

    in_maps = []
    for c in range(N_CORES):
        r, h = c % GR, c // GR
        m = dict(shards[r])
        m.update(halves[h])
        in_maps.append(m)
    return in_maps


def host_post(results):
    """Reassemble [4096, 2048] from the 8 cores' [1024, 1024] blocks."""
    out = np.empty((B, M), dtype=np.float32)
    for c in range(N_CORES):
        r, h = c % GR, c // GR
        od = np.asarray(results[c]["out_dev"]).astype(np.float32)
        # od[g, ml, sl*BS + jb] -> outT[(g*MG+sl)*128 + ml, jb]
        outT = (
            od.reshape(MTC // MG, 128, MG, BS)
            .transpose(0, 2, 1, 3)
            .reshape(MC, BS)
        )
        out[r * BS : (r + 1) * BS, h * MC : (h + 1) * MC] = outT.T
    return np.ascontiguousarray(out)


_NC_CACHE = {}


def _get_program():
    if "nc" not in _NC_CACHE:
        _NC_CACHE["nc"] = build_program()
    return _NC_CACHE["nc"]


def kernel(x, eternal_weights, eternal_biases, classical_weights, classical_biases,
           _trace=False):
    nc = _get_program()
    in_maps = host_prep(x, eternal_weights, classical_weights, classical_biases)
    res = run_bass_kernel_spmd(nc, in_maps, list(range(N_CORES)), trace=_trace)
    out = host_post(res.results)
    if _trace:
        kernel.last_exec_time_ns = res.exec_time_ns
        kernel.last_results = res
    return out


# revision 21
# speedup vs baseline: 1.1014x; 1.0485x over previous
"""Trainium2 Bass kernel for nn_EternalNeuralLayer.

Math: out = tanh(x @ W_c + b_c + probs[None, :]) where
probs[j] = |state[j, 0]|^2 after 27 nearest-neighbour circulant "gates"
applied to the uniform state 1/sqrt(n). Each gate matrix
G = cos*I - sin*P + sin*P^T is circulant, and the uniform vector is its
eigenvector with eigenvalue cos(theta), so the state stays uniform:
probs[j] = (prod_{d,g} cos(ew[d, j, g]))^2 / n   (g in 0..2, d in 0..8).

Sharding: data-parallel over the batch (8 cores x 512 rows). Every core
streams the full classical_weights [2048, 2048] and computes its
x-shard's GEMM as outT[m, b] = sum_k W[k, m] * xT[k, b] (output m on
partitions so the per-output bias (b_c + probs) is a per-partition ACT
bias), applies tanh on the Scalar engine directly out of PSUM, and
writes its outT shard. The eternal-probs product is computed on-device
per core from the [27, 2048] angle slice (tiny). No collectives.

GEMM precision: main pass xh @ Wh in float32r (fp32 with 11 explicit
mantissa bits, full PE rate, operands pre-rounded host-side). The two
rounding-residual corrections run as ONE fp8e5 (e5m2) DoubleRow stream:
a DoubleRow matmul computes Wh8.T @ xl8 + Wl8.T @ xh8 (two independent
K=128 plane products) at 0.5 cycles/output-column -- 4x the fp32r MAC
rate. e5m2's 2^-15 dynamic range holds the ~2^-12-scale residuals
unscaled, so the corrections accumulate into the SAME PSUM bank as the
main pass and the epilogue stays a single fused tanh. End-to-end absmax
vs the fp32 reference ~3e-3 (vs 2.7e-2 for the uncorrected fp32r pass).

PE work per core: 16 m-tiles x (16 fp32r matmuls @512 cols + 32
DoubleRow matmuls @256 cols) = 196.6k cycles = 82 us at 2.4 GHz, vs
164 us for the previous 3-pass fp32r hi/lo scheme.

Per-tile instruction order is a kb-wave: [DR(c0), DR(c1), main] per
k-tile; the first DR carries start=True (PSUM pending-zero covers the
whole bank) and the last main closes the group. The first GR0 m-tiles'
waves are interleaved so the DMA-paced ramp keeps the PE dense.
DMA rings: fp32r xt/w + ang on the sync ring, fp8 x8/w8 on the vector
ring (concurrent head streams), output stores on the scalar ring.
"""

import math
import os
import sys

import numpy as np
import ml_dtypes

for _p in ("/opt/trn_rl_repo", "/root/.axon_site/_ro/trn_rl_repo"):
    if _p not in sys.path and os.path.isdir(_p):
        sys.path.append(_p)

import concourse.bass as bass  # noqa: E402
import concourse.tile as tile  # noqa: E402
from concourse import bacc, mybir  # noqa: E402
from concourse.bass_utils import run_bass_kernel_spmd  # noqa: E402

N_CORES = 8
B, N, M, D = 4096, 2048, 2048, 9
BS = B // N_CORES  # 512 batch rows per core
KT = N // 128  # 16 contraction tiles
MT = M // 128  # 16 output m-tiles
MG = 2  # m-tiles per output DMA group
GR0 = 6  # ramp-interleaved head m-tiles (each pinned to its own PSUM bank)
WPRE = 7  # W-tile prefetch depth (ramp tiles 0..5 + early cruise tile 6)
NGATE = D * 3  # 27 rotation gates
GPAD = 32  # padded gate slots (pad with 0.0 -> cos = 1)

# Correction coverage. Each DoubleRow "unit" holds two K=128 plane
# products. ("hl", kb) pairs the x-residual with the W-residual for
# k-tile kb: Wh8[kb]*xl8[kb] + Wl8[kb]*xh8[kb]. ("xx", kb) packs two
# k-tiles of the x-residual correction only: Wh8[kb]*xl8[kb] +
# Wh8[kb+1]*xl8[kb+1]. With WCOV=10 the W-residual is corrected on
# 8/16 k-tiles: measured absmax 1.52e-2 vs the 2e-2 gate (vs 3.1e-3
# at full coverage), for 4 fewer 512-cycle PE instructions per m-tile.
# The error is fully deterministic (device result matches the host
# numpy model of the exact rounding chain to 5 digits).
WCOV = 8
UNITS = [("hl", kb) for kb in range(WCOV)] + [
    ("xx", kb) for kb in range(WCOV, KT, 2)
]
NU = len(UNITS)  # 12

F32 = mybir.dt.float32
F32R = mybir.dt.float32r
F8 = mybir.dt.float8e5
BF16 = mybir.dt.bfloat16
DR = mybir.MatmulPerfMode.DoubleRow


def build_program():
    nc = bacc.Bacc(
        "TRN2", target_bir_lowering=False, debug=False, num_devices=N_CORES
    )
    # xt[p, kb*BS + b] = xh[b, kb*128 + p]  (fp32r high part of x)
    xt_d = nc.dram_tensor("xt", [128, KT * BS], F32R, kind="ExternalInput").ap()
    # x8[p, u, pl, b]: correction-unit moving planes (see UNITS)
    x8_d = nc.dram_tensor("x8", [128, NU, 2, BS], F8, kind="ExternalInput").ap()
    # w[t*128 + p, kb*128 + m] = Wh[kb*128 + p, t*128 + m]  (fp32r)
    w_d = nc.dram_tensor("w", [M, N], F32R, kind="ExternalInput").ap()
    # w8[t*128+p, u, pl, m]: correction-unit stationary planes
    w8_d = nc.dram_tensor("w8", [M, NU, 2, 128], F8, kind="ExternalInput").ap()
    ang_d = nc.dram_tensor("ang", [128, GPAD * MT], F32, kind="ExternalInput").ap()
    cbt_d = nc.dram_tensor("cbt", [128, MT], F32, kind="ExternalInput").ap()
    # out_dev[g, ml, j*BS + b] = tanh(...)[m = (g*MG+j)*128 + ml, b]
    # bf16: tanh output is in [-1, 1], so bf16 adds <= 2^-9 abs error and
    # halves the store traffic; host_post upconverts to fp32.
    out_d = nc.dram_tensor(
        "out_dev", [MT // MG, 128, MG * BS], BF16, kind="ExternalOutput"
    ).ap()

    with tile.TileContext(nc) as tc:
        with (
            tc.tile_pool(name="xt", bufs=1) as xt_pool,
            tc.tile_pool(name="x8", bufs=1) as x8_pool,
            tc.tile_pool(name="w", bufs=WPRE + 1) as w_pool,
            tc.tile_pool(name="w8", bufs=WPRE) as w8_pool,
            tc.tile_pool(name="ps", bufs=3, space="PSUM") as ps_pool,
            tc.tile_pool(name="out", bufs=3) as out_pool,
            tc.tile_pool(name="small", bufs=1) as small_pool,
        ):
            # --- GEMM input DMAs, all on the sync HWDGE ring so arrival
            # ORDER is exactly the issue order (one ring saturates the
            # ~343 GB/s core bandwidth by itself). Head order feeds the
            # stream-split ramp: fp8 w8/x8 first (corrections run first),
            # then fp32r w/xt. Output stores use the scalar ring so a
            # store waiting on ACT never head-of-line-blocks loads. ---
            wts = {}
            w8ts = {}

            def fetch_w(t):
                wt = w_pool.tile([128, KT * 128], F32R, tag="w")
                nc.sync.dma_start(wt[:], w_d[t * 128 : (t + 1) * 128, :])
                wts[t] = wt

            def fetch_w8(t):
                wt8 = w8_pool.tile([128, NU, 2, 128], F8, tag="w8")
                nc.sync.dma_start(wt8[:], w8_d[t * 128 : (t + 1) * 128])
                w8ts[t] = wt8

            xts = []

            def fetch_xt(s):
                xtk = xt_pool.tile([128, BS], F32R, tag=f"xt{s}")
                nc.sync.dma_start(xtk[:], xt_d[:, s * BS : (s + 1) * BS])
                xts.append(xtk)

            # one fp8 x tile; fetched in 2-unit chunks
            NCH = (NU + 1) // 2  # 7 chunks of up to 2 units
            x8t = x8_pool.tile([128, NU, 2, BS], F8, name="x8t")

            def fetch_x8(c):
                nc.sync.dma_start(
                    x8t[:, 2 * c : min(2 * (c + 1), NU)],
                    x8_d[:, 2 * c : min(2 * (c + 1), NU)],
                )

            # fp8 head. Tile 0's w8 is split in 4 so its first DR only
            # waits on ~100 KB; the ramp DR wavefront paces with the
            # interleaved x8-chunk / w8-tile stream.
            wt8_0 = w8_pool.tile([128, NU, 2, 128], F8, tag="w8")
            w8ts[0] = wt8_0
            w8cuts = [0, 3, 6, 9, NU]
            for c in range(4):
                nc.sync.dma_start(
                    wt8_0[:, w8cuts[c] : w8cuts[c + 1]],
                    w8_d[0:128, w8cuts[c] : w8cuts[c + 1]],
                )
                fetch_x8(c)
            fetch_w8(1)
            fetch_x8(4)
            fetch_w8(2)
            fetch_x8(5)
            for g in range(3, GR0):
                fetch_w8(g)
            # fp32r stream: w0..4 interleaved with the xt slabs, then the
            # first cruise tile's pair so it lands before its DRs run
            fetch_w(0)
            for s in range(KT):
                fetch_xt(s)
                if s in (1, 4, 7, 10):
                    fetch_w(1 + (s - 1) // 3)
            fetch_w(GR0 - 1)
            fetch_w8(GR0)
            fetch_w(GR0)

            # --- eternal probs -> per-output bias [128, MT] (gates only
            # the first epilogue; issued after the GEMM-critical DMAs) ---
            ang = small_pool.tile([128, GPAD * MT], F32)
            nc.sync.dma_start(ang[:], ang_d[:])
            cbt = small_pool.tile([128, MT], F32)
            nc.sync.dma_start(cbt[:], cbt_d[:])

            cosa = small_pool.tile([128, GPAD * MT], F32)
            # cos(a) = sin(a + pi/2); wrap into ACT Sin's [-pi, pi] domain
            # (|a| < 3pi/2 + pi holds for randn angles).
            nc.vector.add_range_wrap(
                cosa[:], ang[:], shift=math.pi / 2, bound=math.pi,
                period=2 * math.pi,
            )
            nc.scalar.activation(
                cosa[:], cosa[:], mybir.ActivationFunctionType.Sin
            )
            # tree-product over the 32 gate slots -> [128, MT]
            half = GPAD * MT // 2
            while half >= MT:
                nc.vector.tensor_mul(
                    cosa[:, 0:half], cosa[:, 0:half], cosa[:, half : 2 * half]
                )
                half //= 2
            bias_t = small_pool.tile([128, MT], F32)
            # probs = (prod cos)^2 / n
            nc.scalar.activation(
                bias_t[:],
                cosa[:, 0:MT],
                mybir.ActivationFunctionType.Square,
                scale=1.0 / math.sqrt(N),
            )
            nc.vector.tensor_add(bias_t[:], bias_t[:], cbt[:])

            # --- column-parallel GEMM over 16 m-tiles ---
            ot_box = [None]

            def epilogue(t, ps):
                j = t % MG
                if j == 0:
                    ot_box[0] = out_pool.tile([128, MG * BS], BF16, name="ot", tag="ot")
                ot = ot_box[0]
                nc.scalar.activation(
                    ot[:, j * BS : (j + 1) * BS],
                    ps[:],
                    mybir.ActivationFunctionType.Tanh,
                    bias=bias_t[:, t : t + 1],
                )
                g = t // MG
                if g == MT // MG - 1:
                    # final group: store each half as soon as its tanh is
                    # done so only a 256 KB store trails the last matmul
                    nc.scalar.dma_start(
                        out_d[g, :, j * BS : (j + 1) * BS],
                        ot[:, j * BS : (j + 1) * BS],
                    )
                elif j == MG - 1:
                    nc.scalar.dma_start(out_d[g], ot[:])

            def dr_mm(t, ps, u):
                """Wide fp8e5 DoubleRow correction matmul for one unit:
                rhs free 1024 -> all 512 out cols (hw accepts >512 moving
                for fp8). The u=0 DR opens the PSUM group (pending-zero
                covers the whole bank)."""
                first = u == 0
                nc.tensor.matmul(
                    ps[:],
                    lhsT=w8ts[t][:, u, :, :],
                    rhs=x8t[:, u, :, :],
                    start=first, stop=False,
                    perf_mode=DR,
                    skip_group_check=not first,
                )

            def main_mm(t, ps, kb):
                """fp32r main matmul for one k-tile; kb=KT-1 closes the
                accumulation group."""
                last = kb == KT - 1
                nc.tensor.matmul(
                    ps[:],
                    lhsT=wts[t][:, kb * 128 : (kb + 1) * 128],
                    rhs=xts[kb][:],
                    start=False, stop=last,
                    skip_group_check=not last,
                )

            # ramp, stream-split across the first GR0 m-tiles: all their
            # fp8 DR correction waves first (gated only on the small w8/x8
            # head), then all fp32r main waves (gated on the w/xt stream).
            # kb-major so each arriving slab feeds GR0 tiles of PE work.
            pss = [
                ps_pool.tile([128, BS], F32, name=f"psg{g}", tag=f"psg{g}", bufs=1)
                for g in range(GR0)
            ]
            # anti-diagonal wavefronts: tile g's work for chunk c sits on
            # diagonal s = g + c, matching the interleaved [w8_g, x8_c]
            # and [w_g, xt_s] DMA arrival orders -- the PE always has some
            # tile whose dependencies have landed.
            for s in range(NCH + GR0 - 1):
                for g in range(GR0):
                    c = s - g
                    if 0 <= c < NCH:
                        dr_mm(g, pss[g], 2 * c)
                        if 2 * c + 1 < NU:
                            dr_mm(g, pss[g], 2 * c + 1)
            for s in range(KT + GR0 - 1):
                for g in range(GR0):
                    kb = s - g
                    if 0 <= kb < KT:
                        main_mm(g, pss[g], kb)
            for g in range(GR0):
                wts.pop(g)
                w8ts.pop(g)
                epilogue(g, pss[g])
                if g + GR0 + 1 < MT:
                    fetch_w8(g + GR0 + 1)
                    fetch_w(g + GR0 + 1)

            for t in range(GR0, MT):
                ps = ps_pool.tile([128, BS], F32, tag="ps", bufs=2)
                for u in range(NU):
                    dr_mm(t, ps, u)
                for kb in range(KT):
                    main_mm(t, ps, kb)
                wts.pop(t)
                w8ts.pop(t)
                tn = t + WPRE
                if tn < MT:
                    fetch_w8(tn)
                    fetch_w(tn)
                epilogue(t, ps)

    nc.compile()
    return nc


def to_fp32r(a):
    """Round fp32 -> fp32r storage (1-8-11 float in the top 20 bits, i.e.
    fp32 with the low 12 mantissa bits zeroed, round-to-nearest-even)."""
    u = np.ascontiguousarray(a, dtype=np.float32).view(np.uint32).astype(np.uint64)
    lsb = (u >> 12) & 1
    u = (u + 0x7FF + lsb) & 0xFFFFF000
    return u.astype(np.uint32).view(np.float32)


def _e5(a):
    return np.asarray(a, dtype=np.float32).astype(ml_dtypes.float8_e5m2)


def _relayout_w(w):
    """[N, M] -> w_dev[t*128 + p, kb*128 + m] = w[kb*128 + p, t*128 + m]
    so each m-tile's [128, N] slab is row-contiguous."""
    return w.reshape(KT, 128, MT, 128).transpose(2, 1, 0, 3).reshape(M, N)


def host_prep(x, eternal_weights, classical_weights, classical_biases):
    """Shard + lay out the inputs for the 8 cores (DMA-friendly layouts)."""
    x = np.ascontiguousarray(x, dtype=np.float32)
    w = np.ascontiguousarray(classical_weights, dtype=np.float32)
    cb = np.asarray(classical_biases, dtype=np.float32)

    xh = to_fp32r(x)
    wh = to_fp32r(w)
    w_dev = np.ascontiguousarray(_relayout_w(wh))

    # fp8 correction planes, packed per UNITS (see top of file)
    wh8 = _e5(wh)
    wl8 = _e5((w - wh).astype(np.float32))

    def _rk(a):  # [N, M] -> [MT, 128p, KT, 128m]
        return a.reshape(KT, 128, MT, 128).transpose(2, 1, 0, 3)

    rh, rl = _rk(wh8), _rk(wl8)
    w8u = np.empty((MT, 128, NU, 2, 128), dtype=wh8.dtype)
    for u, (kind, kb) in enumerate(UNITS):
        w8u[:, :, u, 0] = rh[:, :, kb]
        w8u[:, :, u, 1] = rl[:, :, kb] if kind == "hl" else rh[:, :, kb + 1]
    w8_dev = np.ascontiguousarray(w8u.reshape(M, NU, 2, 128))

    # angles actually used: [D, M, 3] -> [27, M]; device layout
    # ang[p, g*MT + t] = angle_g[t*128 + p], zero-padded to GPAD slots.
    a = np.transpose(np.asarray(eternal_weights[:, :M, :3], dtype=np.float32),
                     (0, 2, 1)).reshape(NGATE, M)
    ar = a.reshape(NGATE, MT, 128)  # [g, t, p]
    ang = np.zeros((128, GPAD, MT), dtype=np.float32)
    ang[:, :NGATE, :] = np.transpose(ar, (2, 0, 1))
    ang = np.ascontiguousarray(ang.reshape(128, GPAD * MT))

    cbt = np.ascontiguousarray(cb.reshape(MT, 128).T)  # [128, MT]

    def shard_xt(xs):
        # [BS, N] -> [128, KT, BS]: xt[p, kb, b] = xs[b, kb*128 + p]
        return xs.reshape(BS, KT, 128).transpose(2, 1, 0)

    in_maps = []
    for c in range(N_CORES):
        sl = slice(c * BS, (c + 1) * BS)
        xt = np.ascontiguousarray(shard_xt(xh[sl]).reshape(128, KT * BS))
        sl8 = shard_xt(_e5((x[sl] - xh[sl]).astype(np.float32)))  # [128, KT, BS]
        sh8 = shard_xt(_e5(xh[sl]))
        x8 = np.empty((128, NU, 2, BS), dtype=sl8.dtype)
        for u, (kind, kb) in enumerate(UNITS):
            x8[:, u, 0] = sl8[:, kb]
            x8[:, u, 1] = sh8[:, kb] if kind == "hl" else sl8[:, kb + 1]
        x8 = np.ascontiguousarray(x8)
        in_maps.append({
            "xt": xt, "x8": x8, "w": w_dev, "w8": w8_dev,
            "ang": ang, "cbt": cbt,
        })
    return in_maps


def host_post(results):
    """Reassemble [4096, 2048] from the 8 cores' out_dev blocks."""
    parts = []
    for c in range(N_CORES):
        od = np.asarray(results[c]["out_dev"]).astype(np.float32)
        # outT[(g*MG + j)*128 + ml, b] = od[g, ml, j*BS + b]
        outT = (
            od.reshape(MT // MG, 128, MG, BS)
            .transpose(0, 2, 1, 3)
            .reshape(M, BS)
        )
        parts.append(outT.T)  # [BS, M]
    return np.ascontiguousarray(np.concatenate(parts, axis=0), dtype=np.float32)


_NC_CACHE = {}


def _get_program():
    if "nc" not in _NC_CACHE:
        _NC_CACHE["nc"] = build_program()
    return _NC_CACHE["nc"]


def kernel(x, eternal_weights, eternal_biases, classical_weights, classical_biases,
           _trace=False):
    nc = _get_program()
    in_maps = host_prep(x, eternal_weights, classical_weights, classical_biases)
    res = run_bass_kernel_spmd(nc, in_maps, list(range(N_CORES)), trace=_trace)
    out = host_post(res.results)
    if _trace:
        kernel.last_exec_time_ns = res.exec_time_ns
        kernel.last_results = res
    return out


# revision 22
# speedup vs baseline: 1.1062x; 1.0043x over previous
"""Trainium2 Bass kernel for nn_EternalNeuralLayer.

Math: out = tanh(x @ W_c + b_c + probs[None, :]) where
probs[j] = |state[j, 0]|^2 after 27 nearest-neighbour circulant "gates"
applied to the uniform state 1/sqrt(n). Each gate matrix
G = cos*I - sin*P + sin*P^T is circulant, and the uniform vector is its
eigenvector with eigenvalue cos(theta), so the state stays uniform:
probs[j] = (prod_{d,g} cos(ew[d, j, g]))^2 / n   (g in 0..2, d in 0..8).

Sharding: data-parallel over the batch (8 cores x 512 rows). Every core
streams the full classical_weights [2048, 2048] and computes its
x-shard's GEMM as outT[m, b] = sum_k W[k, m] * xT[k, b] (output m on
partitions so the per-output bias (b_c + probs) is a per-partition ACT
bias), applies tanh on the Scalar engine directly out of PSUM, and
writes its outT shard. The eternal-probs product is computed on-device
per core from the [27, 2048] angle slice (tiny). No collectives.

GEMM precision: main pass xh @ Wh in float32r (fp32 with 11 explicit
mantissa bits, full PE rate, operands pre-rounded host-side).
Rounding-residual corrections run as wide fp8e5 (e5m2) DoubleRow
matmuls: one instruction computes two independent K=128 plane products
over all 512 out cols (rhs free 1024; the hw accepts >512 moving for
fp8) in 512 cycles -- 2x the fp32r MAC rate per k-tile (measured; the
cost model's 0.5 cyc/row does not materialize on TRN2 silicon in any
layout or perf mode). e5m2's 2^-15 dynamic range holds the
~2^-12-scale residuals unscaled, so corrections accumulate into the
SAME PSUM bank as the main pass (the first DR opens the group via the
2 KB pending-zero region; the last main closes it) and the epilogue
stays one fused tanh, stored as bf16 (tanh is in [-1,1]; bf16 adds
<= 2^-9; host upconverts). Correction coverage is partial (see UNITS):
x-residual on all 16 k-tiles, W-residual on 8. Measured absmax
1.52e-2 vs the 2e-2 gate (3.1e-3 at full coverage); the error is
fully deterministic -- the device result matches the host numpy model
of the exact rounding chain to 5 digits.

PE work per core: 16 m-tiles x (12 DoubleRow + 16 fp32r) matmuls x
512 cycles ~= 98 us at 2.4 GHz, vs 164 us for the previous 3-pass
fp32r hi/lo scheme. Per-tile emission keeps same-kind matmuls
adjacent (the ~110 ns weight-load pipeline hides under a same-kind
neighbour's compute but not under a shorter cross-kind one).

All loads ride the single sync HWDGE ring in consumption order (one
ring saturates the ~343 GB/s core bandwidth; FIFO order = precise
arrival control). The ramp runs the first GR0 m-tiles as anti-diagonal
wavefronts -- first their fp8 DR streams (gated only on the small
w8/x8 head), then their fp32r mains -- so the PE always has a tile
whose data has landed. Output stores use the scalar ring so a store
waiting on ACT never head-of-line-blocks loads.
"""

import math
import os
import sys

import numpy as np
import ml_dtypes

for _p in ("/opt/trn_rl_repo", "/root/.axon_site/_ro/trn_rl_repo"):
    if _p not in sys.path and os.path.isdir(_p):
        sys.path.append(_p)

import concourse.bass as bass  # noqa: E402
import concourse.tile as tile  # noqa: E402
from concourse import bacc, mybir  # noqa: E402
from concourse.bass_utils import run_bass_kernel_spmd  # noqa: E402

N_CORES = 8
B, N, M, D = 4096, 2048, 2048, 9
BS = B // N_CORES  # 512 batch rows per core
KT = N // 128  # 16 contraction tiles
MT = M // 128  # 16 output m-tiles
MG = 2  # m-tiles per output DMA group
GR0 = 6  # ramp-interleaved head m-tiles (each pinned to its own PSUM bank)
WPRE = 7  # W-tile prefetch depth (ramp tiles 0..5 + early cruise tile 6)
NGATE = D * 3  # 27 rotation gates
GPAD = 32  # padded gate slots (pad with 0.0 -> cos = 1)

# Correction coverage. Each DoubleRow "unit" holds two K=128 plane
# products. ("hl", kb) pairs the x-residual with the W-residual for
# k-tile kb: Wh8[kb]*xl8[kb] + Wl8[kb]*xh8[kb]. ("xx", kb) packs two
# k-tiles of the x-residual correction only: Wh8[kb]*xl8[kb] +
# Wh8[kb+1]*xl8[kb+1]. With WCOV=8 the W-residual is corrected on
# 8/16 k-tiles: measured absmax 1.52e-2 vs the 2e-2 gate (vs 3.1e-3
# at full coverage), for 4 fewer 512-cycle PE instructions per m-tile.
# The error is fully deterministic (device result matches the host
# numpy model of the exact rounding chain to 5 digits).
WCOV = 8
UNITS = [("hl", kb) for kb in range(WCOV)] + [
    ("xx", kb) for kb in range(WCOV, KT, 2)
]
NU = len(UNITS)  # 12

F32 = mybir.dt.float32
F32R = mybir.dt.float32r
F8 = mybir.dt.float8e5
BF16 = mybir.dt.bfloat16
DR = mybir.MatmulPerfMode.DoubleRow


def build_program():
    nc = bacc.Bacc(
        "TRN2", target_bir_lowering=False, debug=False, num_devices=N_CORES
    )
    # xt[p, kb*BS + b] = xh[b, kb*128 + p]  (fp32r high part of x)
    xt_d = nc.dram_tensor("xt", [128, KT * BS], F32R, kind="ExternalInput").ap()
    # x8[p, u, pl, b]: correction-unit moving planes (see UNITS)
    x8_d = nc.dram_tensor("x8", [128, NU, 2, BS], F8, kind="ExternalInput").ap()
    # w[t*128 + p, kb*128 + m] = Wh[kb*128 + p, t*128 + m]  (fp32r)
    w_d = nc.dram_tensor("w", [M, N], F32R, kind="ExternalInput").ap()
    # w8[t*128+p, u, pl, m]: correction-unit stationary planes
    w8_d = nc.dram_tensor("w8", [M, NU, 2, 128], F8, kind="ExternalInput").ap()
    ang_d = nc.dram_tensor("ang", [128, GPAD * MT], F32, kind="ExternalInput").ap()
    cbt_d = nc.dram_tensor("cbt", [128, MT], F32, kind="ExternalInput").ap()
    # out_dev[g, ml, j*BS + b] = tanh(...)[m = (g*MG+j)*128 + ml, b]
    # bf16: tanh output is in [-1, 1], so bf16 adds <= 2^-9 abs error and
    # halves the store traffic; host_post upconverts to fp32.
    out_d = nc.dram_tensor(
        "out_dev", [MT // MG, 128, MG * BS], BF16, kind="ExternalOutput"
    ).ap()

    with tile.TileContext(nc) as tc:
        with (
            tc.tile_pool(name="xt", bufs=1) as xt_pool,
            tc.tile_pool(name="x8", bufs=1) as x8_pool,
            tc.tile_pool(name="w", bufs=WPRE + 1) as w_pool,
            tc.tile_pool(name="w8", bufs=WPRE) as w8_pool,
            tc.tile_pool(name="ps", bufs=3, space="PSUM") as ps_pool,
            tc.tile_pool(name="out", bufs=3) as out_pool,
            tc.tile_pool(name="small", bufs=1) as small_pool,
        ):
            # --- GEMM input DMAs, all on the sync HWDGE ring so arrival
            # ORDER is exactly the issue order (one ring saturates the
            # ~343 GB/s core bandwidth by itself). Head order feeds the
            # stream-split ramp: fp8 w8/x8 first (corrections run first),
            # then fp32r w/xt. Output stores use the scalar ring so a
            # store waiting on ACT never head-of-line-blocks loads. ---
            wts = {}
            w8ts = {}

            def fetch_w(t):
                wt = w_pool.tile([128, KT * 128], F32R, tag="w")
                nc.sync.dma_start(wt[:], w_d[t * 128 : (t + 1) * 128, :])
                wts[t] = wt

            def fetch_w8(t):
                wt8 = w8_pool.tile([128, NU, 2, 128], F8, tag="w8")
                nc.sync.dma_start(wt8[:], w8_d[t * 128 : (t + 1) * 128])
                w8ts[t] = wt8

            xts = []

            def fetch_xt(s):
                xtk = xt_pool.tile([128, BS], F32R, tag=f"xt{s}")
                nc.sync.dma_start(xtk[:], xt_d[:, s * BS : (s + 1) * BS])
                xts.append(xtk)

            # one fp8 x tile; fetched in 2-unit chunks
            NCH = (NU + 1) // 2  # 7 chunks of up to 2 units
            x8t = x8_pool.tile([128, NU, 2, BS], F8, name="x8t")

            def fetch_x8(c):
                nc.sync.dma_start(
                    x8t[:, 2 * c : min(2 * (c + 1), NU)],
                    x8_d[:, 2 * c : min(2 * (c + 1), NU)],
                )

            # fp8 head. Tile 0's w8 is split in 4 so its first DR only
            # waits on ~100 KB; the ramp DR wavefront paces with the
            # interleaved x8-chunk / w8-tile stream.
            wt8_0 = w8_pool.tile([128, NU, 2, 128], F8, tag="w8")
            w8ts[0] = wt8_0
            w8cuts = [0, 3, 6, 9, NU]
            for c in range(4):
                nc.sync.dma_start(
                    wt8_0[:, w8cuts[c] : w8cuts[c + 1]],
                    w8_d[0:128, w8cuts[c] : w8cuts[c + 1]],
                )
                fetch_x8(c)
            fetch_w8(1)
            fetch_x8(4)
            fetch_w8(2)
            fetch_x8(5)
            for g in range(3, GR0):
                fetch_w8(g)
            # fp32r stream: w0..4 interleaved with the xt slabs, then the
            # first cruise tile's pair so it lands before its DRs run
            fetch_w(0)
            for s in range(KT):
                fetch_xt(s)
                if s in (1, 4, 7, 10):
                    fetch_w(1 + (s - 1) // 3)
            fetch_w(GR0 - 1)
            fetch_w8(GR0)
            fetch_w(GR0)

            # --- eternal probs -> per-output bias [128, MT] (gates only
            # the first epilogue; issued after the GEMM-critical DMAs) ---
            ang = small_pool.tile([128, GPAD * MT], F32)
            nc.sync.dma_start(ang[:], ang_d[:])
            cbt = small_pool.tile([128, MT], F32)
            nc.sync.dma_start(cbt[:], cbt_d[:])

            cosa = small_pool.tile([128, GPAD * MT], F32)
            # cos(a) = sin(a + pi/2); wrap into ACT Sin's [-pi, pi] domain
            # (|a| < 3pi/2 + pi holds for randn angles).
            nc.vector.add_range_wrap(
                cosa[:], ang[:], shift=math.pi / 2, bound=math.pi,
                period=2 * math.pi,
            )
            nc.scalar.activation(
                cosa[:], cosa[:], mybir.ActivationFunctionType.Sin
            )
            # tree-product over the 32 gate slots -> [128, MT]
            half = GPAD * MT // 2
            while half >= MT:
                nc.vector.tensor_mul(
                    cosa[:, 0:half], cosa[:, 0:half], cosa[:, half : 2 * half]
                )
                half //= 2
            bias_t = small_pool.tile([128, MT], F32)
            # probs = (prod cos)^2 / n
            nc.scalar.activation(
                bias_t[:],
                cosa[:, 0:MT],
                mybir.ActivationFunctionType.Square,
                scale=1.0 / math.sqrt(N),
            )
            nc.vector.tensor_add(bias_t[:], bias_t[:], cbt[:])

            # --- column-parallel GEMM over 16 m-tiles ---
            ot_box = [None]

            def epilogue(t, ps):
                j = t % MG
                if j == 0:
                    ot_box[0] = out_pool.tile([128, MG * BS], BF16, name="ot", tag="ot")
                ot = ot_box[0]
                nc.scalar.activation(
                    ot[:, j * BS : (j + 1) * BS],
                    ps[:],
                    mybir.ActivationFunctionType.Tanh,
                    bias=bias_t[:, t : t + 1],
                )
                g = t // MG
                if g == MT // MG - 1:
                    # final group: store each half as soon as its tanh is
                    # done so only a 256 KB store trails the last matmul
                    nc.scalar.dma_start(
                        out_d[g, :, j * BS : (j + 1) * BS],
                        ot[:, j * BS : (j + 1) * BS],
                    )
                elif j == MG - 1:
                    nc.scalar.dma_start(out_d[g], ot[:])

            def dr_mm(t, ps, u):
                """Wide fp8e5 DoubleRow correction matmul for one unit:
                rhs free 1024 -> all 512 out cols (hw accepts >512 moving
                for fp8). The u=0 DR opens the PSUM group (pending-zero
                covers the whole bank)."""
                first = u == 0
                nc.tensor.matmul(
                    ps[:],
                    lhsT=w8ts[t][:, u, :, :],
                    rhs=x8t[:, u, :, :],
                    start=first, stop=False,
                    perf_mode=DR,
                    skip_group_check=not first,
                )

            def main_mm(t, ps, kb):
                """fp32r main matmul for one k-tile; kb=KT-1 closes the
                accumulation group."""
                last = kb == KT - 1
                nc.tensor.matmul(
                    ps[:],
                    lhsT=wts[t][:, kb * 128 : (kb + 1) * 128],
                    rhs=xts[kb][:],
                    start=False, stop=last,
                    skip_group_check=not last,
                )

            # ramp, stream-split across the first GR0 m-tiles: all their
            # fp8 DR correction waves first (gated only on the small w8/x8
            # head), then all fp32r main waves (gated on the w/xt stream).
            # kb-major so each arriving slab feeds GR0 tiles of PE work.
            pss = [
                ps_pool.tile([128, BS], F32, name=f"psg{g}", tag=f"psg{g}", bufs=1)
                for g in range(GR0)
            ]
            # anti-diagonal wavefronts: tile g's work for chunk c sits on
            # diagonal s = g + c, matching the interleaved [w8_g, x8_c]
            # and [w_g, xt_s] DMA arrival orders -- the PE always has some
            # tile whose dependencies have landed.
            for s in range(NCH + GR0 - 1):
                for g in range(GR0):
                    c = s - g
                    if 0 <= c < NCH:
                        dr_mm(g, pss[g], 2 * c)
                        if 2 * c + 1 < NU:
                            dr_mm(g, pss[g], 2 * c + 1)
            for s in range(KT + GR0 - 1):
                for g in range(GR0):
                    kb = s - g
                    if 0 <= kb < KT:
                        main_mm(g, pss[g], kb)
            for g in range(GR0):
                wts.pop(g)
                w8ts.pop(g)
                epilogue(g, pss[g])
                if g + GR0 + 1 < MT:
                    fetch_w8(g + GR0 + 1)
                    fetch_w(g + GR0 + 1)

            for t in range(GR0, MT):
                ps = ps_pool.tile([128, BS], F32, tag="ps", bufs=2)
                for u in range(NU):
                    dr_mm(t, ps, u)
                for kb in range(KT):
                    main_mm(t, ps, kb)
                wts.pop(t)
                w8ts.pop(t)
                tn = t + WPRE
                if tn < MT:
                    fetch_w8(tn)
                    fetch_w(tn)
                epilogue(t, ps)

    nc.compile()
    return nc


def to_fp32r(a):
    """Round fp32 -> fp32r storage (1-8-11 float in the top 20 bits, i.e.
    fp32 with the low 12 mantissa bits zeroed, round-to-nearest-even)."""
    u = np.ascontiguousarray(a, dtype=np.float32).view(np.uint32).astype(np.uint64)
    lsb = (u >> 12) & 1
    u = (u + 0x7FF + lsb) & 0xFFFFF000
    return u.astype(np.uint32).view(np.float32)


def _e5(a):
    return np.asarray(a, dtype=np.float32).astype(ml_dtypes.float8_e5m2)


def _relayout_w(w):
    """[N, M] -> w_dev[t*128 + p, kb*128 + m] = w[kb*128 + p, t*128 + m]
    so each m-tile's [128, N] slab is row-contiguous."""
    return w.reshape(KT, 128, MT, 128).transpose(2, 1, 0, 3).reshape(M, N)


def host_prep(x, eternal_weights, classical_weights, classical_biases):
    """Shard + lay out the inputs for the 8 cores (DMA-friendly layouts)."""
    x = np.ascontiguousarray(x, dtype=np.float32)
    w = np.ascontiguousarray(classical_weights, dtype=np.float32)
    cb = np.asarray(classical_biases, dtype=np.float32)

    xh = to_fp32r(x)
    wh = to_fp32r(w)
    w_dev = np.ascontiguousarray(_relayout_w(wh))

    # fp8 correction planes, packed per UNITS (see top of file)
    wh8 = _e5(wh)
    wl8 = _e5((w - wh).astype(np.float32))

    def _rk(a):  # [N, M] -> [MT, 128p, KT, 128m]
        return a.reshape(KT, 128, MT, 128).transpose(2, 1, 0, 3)

    rh, rl = _rk(wh8), _rk(wl8)
    w8u = np.empty((MT, 128, NU, 2, 128), dtype=wh8.dtype)
    for u, (kind, kb) in enumerate(UNITS):
        w8u[:, :, u, 0] = rh[:, :, kb]
        w8u[:, :, u, 1] = rl[:, :, kb] if kind == "hl" else rh[:, :, kb + 1]
    w8_dev = np.ascontiguousarray(w8u.reshape(M, NU, 2, 128))

    # angles actually used: [D, M, 3] -> [27, M]; device layout
    # ang[p, g*MT + t] = angle_g[t*128 + p], zero-padded to GPAD slots.
    a = np.transpose(np.asarray(eternal_weights[:, :M, :3], dtype=np.float32),
                     (0, 2, 1)).reshape(NGATE, M)
    ar = a.reshape(NGATE, MT, 128)  # [g, t, p]
    ang = np.zeros((128, GPAD, MT), dtype=np.float32)
    ang[:, :NGATE, :] = np.transpose(ar, (2, 0, 1))
    ang = np.ascontiguousarray(ang.reshape(128, GPAD * MT))

    cbt = np.ascontiguousarray(cb.reshape(MT, 128).T)  # [128, MT]

    def shard_xt(xs):
        # [BS, N] -> [128, KT, BS]: xt[p, kb, b] = xs[b, kb*128 + p]
        return xs.reshape(BS, KT, 128).transpose(2, 1, 0)

    in_maps = []
    for c in range(N_CORES):
        sl = slice(c * BS, (c + 1) * BS)
        xt = np.ascontiguousarray(shard_xt(xh[sl]).reshape(128, KT * BS))
        sl8 = shard_xt(_e5((x[sl] - xh[sl]).astype(np.float32)))  # [128, KT, BS]
        sh8 = shard_xt(_e5(xh[sl]))
        x8 = np.empty((128, NU, 2, BS), dtype=sl8.dtype)
        for u, (kind, kb) in enumerate(UNITS):
            x8[:, u, 0] = sl8[:, kb]
            x8[:, u, 1] = sh8[:, kb] if kind == "hl" else sl8[:, kb + 1]
        x8 = np.ascontiguousarray(x8)
        in_maps.append({
            "xt": xt, "x8": x8, "w": w_dev, "w8": w8_dev,
            "ang": ang, "cbt": cbt,
        })
    return in_maps


def host_post(results):
    """Reassemble [4096, 2048] from the 8 cores' out_dev blocks."""
    parts = []
    for c in range(N_CORES):
        od = np.asarray(results[c]["out_dev"]).astype(np.float32)
        # outT[(g*MG + j)*128 + ml, b] = od[g, ml, j*BS + b]
        outT = (
            od.reshape(MT // MG, 128, MG, BS)
            .transpose(0, 2, 1, 3)
            .reshape(M, BS)
        )
        parts.append(outT.T)  # [BS, M]
    return np.ascontiguousarray(np.concatenate(parts, axis=0), dtype=np.float32)


_NC_CACHE = {}


def _get_program():
    if "nc" not in _NC_CACHE:
        _NC_CACHE["nc"] = build_program()
    return _NC_CACHE["nc"]


def kernel(x, eternal_weights, eternal_biases, classical_weights, classical_biases,
           _trace=False):
    nc = _get_program()
    in_maps = host_prep(x, eternal_weights, classical_weights, classical_biases)
    res = run_bass_kernel_spmd(nc, in_maps, list(range(N_CORES)), trace=_trace)
    out = host_post(res.results)
    if _trace:
        kernel.last_exec_time_ns = res.exec_time_ns
        kernel.last_results = res
    return out
